# revision 30
# baseline (speedup 1.0000x reference)
"""Trainium2 Bass kernel for nn_DechunkingLayer (ragged_sequence).

Reference semantics (per batch row):
    idx = clip(exclusive_cumsum(b), 0, NC - 1)          # [T]
    up[t]  = z[idx[t]]                                  # gather rows
    out[t] = p[t] * up[t] + (1 - p[t]) * up[t-1]        # EMA blend
    out[0] = up[0]

Sharding: pure data parallel over batch B=8 across the 8 NeuronCores
(one batch row per core). All work per row is independent.

Per-core plan (HBM traffic = 16 MB fp32 gather + 8 MB bf16 store):
  - exclusive cumsum of the 0/1 boundary flags computed on-device with a
    PE triangular-matmul scan in a [128, 32] "W layout" (partition = t % 128,
    column = t // 128) — exactly the layout the indirect-DMA gather wants
    its per-partition row indices in. Tile-0 indices take a short path so
    the first gather issues before the column-offset chain finishes.
  - rolled (up[t-1]) is the gathered tile shifted down one partition.
    Compute engines cannot read partition-shifted operands (windows must
    be 32-aligned), and SBUF->SBUF DMA shifts eat DMA-queue bandwidth
    (the binding resource: ~25 GB/s per queue x 16 queues), so the shift
    rides the PE as a shifted-identity fp32 matmul (bitwise exact).
    fp32 (4 cycles/row) is mandatory: fp32r/bf16 round the streamed data,
    which blows up relative error at cancellation points of the blend.
  - NO PE warmup: the HAM clock gate throttles the PE until ~4us of
    cumulative busy, but warmup matmuls delay the cumsum chain / first
    gather more than the brief half-clock ramp costs.
  - the final store is bf16: rounding the FINAL value keeps max relative
    error <= 2^-9 even under cancellation (rounding any blend INPUT would
    not); host upcasts to fp32. Halves store traffic.
  - per-tile rows t=128k blend against the previous tile's last row; those
    32 rows are redone exactly in a small epilogue pass (2 gathers of 32
    rows + blend) stored to a SEPARATE output tensor (no write-after-write
    wait on the 32 main stores, which would serialize the tail); the host
    merges the rows back (pure assembly, no arithmetic).
  - out[0] = up[0] exactly via forcing p[0] = 1 (q[0] = 0).
"""

import numpy as np

import concourse.bacc as bacc
import concourse.bass as bass
import concourse.mybir as mybir
import concourse.tile as tile
from concourse.bass import IndirectOffsetOnAxis
from concourse.bass_utils import run_bass_kernel_spmd
from concourse.masks import make_identity, make_upper_triangular

# Problem shape (hardcoded per harness contract).
B = 8          # batch rows == number of cores
T = 4096       # timesteps per row
NCH = 2048     # number of chunks (z rows)
D = 1024       # d_model
P = 128        # SBUF partitions
NT = T // P    # 32 tiles per core
NCOL = T // P  # 32 columns in the W layout
DH = D // 2    # matmul free-dim max for fp32 is 512

F32 = mybir.dt.float32
F32R = mybir.dt.float32r
BF16 = mybir.dt.bfloat16
I32 = mybir.dt.int32

# Measured dead ends, kept disabled: a second HBM gather for `rolled`
# (GATHER_STRIDE) loses to the PE shift; a partition-shifted SBUF->SBUF
# DMA (SHIFT_STRIDE) eats DMA-queue bytes, the binding resource.
GATHER_STRIDE = 1000  # > NT: disabled
SHIFT_STRIDE = 1000   # > NT: disabled
WARMUP_MM = 5         # unused (warmup removed; loop self-warms the PE)


def build_bass() -> bass.Bass:
    # Bacc (not raw Bass): its finalize() runs generate_event_semaphores,
    # which splits multi-sem waits to satisfy TRN2's one-wait-per-instruction
    # ISA constraint.
    nc = bacc.Bacc()

    z = nc.dram_tensor("z", [NCH, D], F32, kind="ExternalInput")
    p = nc.dram_tensor("p", [T], F32, kind="ExternalInput")
    b = nc.dram_tensor("b", [T], I32, kind="ExternalInput")
    out = nc.dram_tensor("out", [T, D], BF16, kind="ExternalOutput")
    # rows t = 128j are redone exactly; they go to a SEPARATE tensor so
    # their store has no write-after-write dependency on the 32 main
    # stores (which would serialize the tail); the host merges them.
    out2 = nc.dram_tensor("out2", [NCOL, D], BF16, kind="ExternalOutput")

    with tile.TileContext(nc) as tc:
        with (
            tc.tile_pool(name="setup", bufs=1) as sp,
            tc.tile_pool(name="psmall", bufs=2, space="PSUM") as pps,
            tc.tile_pool(name="proll", bufs=3, space="PSUM") as ppr,
            tc.tile_pool(name="main", bufs=10) as mp,
        ):
            # ---- constants -------------------------------------------------
            # affine_select only exists on gpsimd; PE Matmult has a single
            # sync-wait slot, so launder every matmul operand through DVE so
            # all matmul waits collapse onto one DVE semaphore.
            tri_g = sp.tile([P, P], F32)     # tri[k, i] = 1 iff i > k
            make_upper_triangular(nc, tri_g[:], val=1.0, diag=False)
            tri = sp.tile([P, P], F32)
            nc.vector.tensor_copy(out=tri[:], in_=tri_g[:])

            ident_g = sp.tile([NCOL, NCOL], F32)
            make_identity(nc, ident_g[:])
            ident = sp.tile([NCOL, NCOL], F32)
            nc.vector.tensor_copy(out=ident[:], in_=ident_g[:])

            tri32_g = sp.tile([NCOL, NCOL], F32)  # [k, j] = 1 iff j > k
            make_upper_triangular(nc, tri32_g[:], val=1.0, diag=False)
            tri32 = sp.tile([NCOL, NCOL], F32)
            nc.vector.tensor_copy(out=tri32[:], in_=tri32_g[:])

            # shifted identity: S[k, i] = 1 iff i == k + 1  ->  (S^T @ x)[i] = x[i-1]
            ish_g = sp.tile([P, P], F32)
            nc.gpsimd.memset(ish_g[:], 0.0)
            nc.gpsimd.affine_select(
                out=ish_g[:], in_=ish_g[:],
                compare_op=mybir.AluOpType.not_equal, fill=1.0,
                base=1, pattern=[[-1, P]], channel_multiplier=1,
            )
            ishift = sp.tile([P, P], F32)
            nc.vector.tensor_copy(out=ishift[:], in_=ish_g[:])

            ones_row = sp.tile([1, P], F32)  # lhsT for partition-broadcast
            nc.vector.memset(ones_row[:], 1.0)
            ones_col = sp.tile([P, 1], F32)  # lhsT for column sums
            nc.vector.memset(ones_col[:], 1.0)


            # ---- load b and p in natural [32, 128] layout ------------------
            b2d = b[:].rearrange("(j c) -> j c", c=P)          # [32, 128] DRAM view
            p2d = p[:].rearrange("(j c) -> j c", c=P)

            b_nat_i = sp.tile([NCOL, P], I32)
            nc.sync.dma_start(out=b_nat_i[:], in_=b2d)
            p_nat = sp.tile([NCOL, P], F32)
            nc.sync.dma_start(out=p_nat[:], in_=p2d)

            b_nat = sp.tile([NCOL, P], F32)
            nc.vector.tensor_copy(out=b_nat[:], in_=b_nat_i[:])

            # b_shifted[t] = b[t-1] (0 at t=0) for idx_prev of the gather-tiles
            use_gather_tiles = False  # PE shift is fast; no tail trim needed
            if use_gather_tiles:
                bp_nat_i = sp.tile([NCOL, P], I32)
                nc.vector.memset(bp_nat_i[0:1, 0:1], 0)
                nc.sync.dma_start(out=bp_nat_i[:, 1:P], in_=b2d[:, 0 : P - 1])
                nc.sync.dma_start(
                    out=bp_nat_i[1:NCOL, 0:1], in_=b2d[0 : NCOL - 1, P - 1 : P]
                )
                bp_nat = sp.tile([NCOL, P], F32)
                nc.vector.tensor_copy(out=bp_nat[:], in_=bp_nat_i[:])

            # ---- PE transpose to W layout [128, 32]: (p, j) = t = 128j + p --
            bw_ps = pps.tile([P, NCOL], F32, space="PSUM", tag="small_ps")
            nc.tensor.transpose(out=bw_ps[:], in_=b_nat[:], identity=ident[:])
            b_w = sp.tile([P, NCOL], F32)
            nc.vector.tensor_copy(out=b_w[:], in_=bw_ps[:])

            # tile-0 indices on a short path: colofs[0] = 0, so column 0
            # needs only the partition scan — the first gather can issue
            # before the column-offset chain finishes.
            s0_ps = pps.tile([P, 1], F32, space="PSUM", tag="small_ps")
            nc.tensor.matmul(out=s0_ps[:], lhsT=tri[:], rhs=b_w[:, 0:1],
                             start=True, stop=True)
            idx0_f = sp.tile([P, 1], F32)
            nc.vector.tensor_scalar_min(out=idx0_f[:], in0=s0_ps[:],
                                        scalar1=float(NCH - 1))
            idx0_i = sp.tile([P, 1], I32)
            nc.vector.tensor_copy(out=idx0_i[:], in_=idx0_f[:])

            if use_gather_tiles:
                bpw_ps = pps.tile([P, NCOL], F32, space="PSUM", tag="small_ps")
                nc.tensor.transpose(out=bpw_ps[:], in_=bp_nat[:], identity=ident[:])
                bp_w = sp.tile([P, NCOL], F32)
                nc.vector.tensor_copy(out=bp_w[:], in_=bpw_ps[:])

            pw_ps = pps.tile([P, NCOL], F32, space="PSUM", tag="small_ps")
            nc.tensor.transpose(out=pw_ps[:], in_=p_nat[:], identity=ident[:])
            p_w = sp.tile([P, NCOL], F32)
            nc.vector.tensor_copy(out=p_w[:], in_=pw_ps[:])
            # out[0] = up[0] exactly: force p[0] = 1 so the blend is 1*up + 0*rolled
            nc.vector.memset(p_w[0:1, 0:1], 1.0)
            q_w = sp.tile([P, NCOL], F32)  # q = 1 - p
            nc.scalar.activation(
                out=q_w[:], in_=p_w[:],
                func=mybir.ActivationFunctionType.Copy, bias=1.0, scale=-1.0,
            )

            # ---- column offsets via two PE matmuls -------------------------
            # tot_col[j'] = sum_k b_w[k, j'] as a column, then
            # colofs[0, j] = sum_{j'<j} tot[j'] via the strict triangular.
            totc_ps = pps.tile([NCOL, 1], F32, space="PSUM", tag="small_ps")
            nc.tensor.matmul(out=totc_ps[:], lhsT=b_w[:], rhs=ones_col[:],
                             start=True, stop=True)
            tot_col = sp.tile([NCOL, 1], F32)
            nc.vector.tensor_copy(out=tot_col[:], in_=totc_ps[:])
            cofs_ps = pps.tile([1, NCOL], F32, space="PSUM", tag="small_ps")
            nc.tensor.matmul(out=cofs_ps[:], lhsT=tot_col[:], rhs=tri32[:],
                             start=True, stop=True)
            colofs = sp.tile([1, NCOL], F32)
            nc.vector.tensor_copy(out=colofs[:], in_=cofs_ps[:])

            # ---- full exclusive cumsum s[t] in W layout --------------------
            # s_ps[i, j] = sum_{k<i} b_w[k, j]  +  colofs[j]
            s_ps = pps.tile([P, NCOL], F32, space="PSUM", tag="small_ps")
            nc.tensor.matmul(out=s_ps[:], lhsT=tri[:], rhs=b_w[:],
                             start=True, stop=False)
            nc.tensor.matmul(out=s_ps[:], lhsT=ones_row[:], rhs=colofs[:],
                             start=False, stop=True)

            # ---- gather indices: idx = min(s, NCH-1) -----------------------
            idx_f = sp.tile([P, NCOL], F32)
            nc.vector.tensor_scalar_min(out=idx_f[:], in0=s_ps[:], scalar1=float(NCH - 1))
            idx_i = sp.tile([P, NCOL], I32)
            nc.vector.tensor_copy(out=idx_i[:], in_=idx_f[:])

            # idx_prev = min(s - b_shifted, NCH-1)  (s[t] - b[t-1] = s[t-1])
            if use_gather_tiles:
                sprev_f = sp.tile([P, NCOL], F32)
                nc.vector.tensor_sub(out=sprev_f[:], in0=s_ps[:], in1=bp_w[:])
                idxp_f = sp.tile([P, NCOL], F32)
                nc.vector.tensor_scalar_min(
                    out=idxp_f[:], in0=sprev_f[:], scalar1=float(NCH - 1)
                )
                idxp_i = sp.tile([P, NCOL], I32)
                nc.vector.tensor_copy(out=idxp_i[:], in_=idxp_f[:])

            # ---- epilogue vectors for rows t = 128j ------------------------
            # bprev_row[j] = idx[128j - 1] (0 for j=0, harmless: q[0]=0).
            # Row 127 of idx_f is not a legal compute-engine base, so extract
            # it with a tiny SBUF->SBUF DMA, then rotate rows into columns
            # with [1,32]-lhsT matmuls against a single 1.0.
            bprev_row = sp.tile([1, NCOL], F32)
            nc.vector.memset(bprev_row[:], 0.0)
            nc.sync.dma_start(
                out=bprev_row[0:1, 1:NCOL], in_=idx_f[P - 1 : P, 0 : NCOL - 1]
            )

            cols_ps = pps.tile([NCOL, 4], F32, space="PSUM", tag="small_ps")
            for ci, row in enumerate([bprev_row, idx_f, p_w, q_w]):
                nc.tensor.matmul(
                    out=cols_ps[:, ci : ci + 1],
                    lhsT=row[0:1, 0:NCOL],
                    rhs=ones_row[0:1, 0:1],
                    start=True, stop=True,
                )
            bidx_i = sp.tile([NCOL, 1], I32)
            nc.vector.tensor_copy(out=bidx_i[:], in_=cols_ps[:, 0:1])
            fidx_i = sp.tile([NCOL, 1], I32)
            nc.vector.tensor_copy(out=fidx_i[:], in_=cols_ps[:, 1:2])
            pb_col = sp.tile([NCOL, 1], F32)
            nc.vector.tensor_copy(out=pb_col[:], in_=cols_ps[:, 2:3])
            qb_col = sp.tile([NCOL, 1], F32)
            nc.vector.tensor_copy(out=qb_col[:], in_=cols_ps[:, 3:4])

            # ---- main loop: gather, roll, blend, store ---------------------
            # The roll (rolled[i] = up[i-1]) costs either PE time (shifted-
            # identity matmul, exact; fp32 runs HI/LO = 2 passes) or HBM
            # bandwidth (a second gather). Neither engine can absorb all 32
            # tiles without becoming the bottleneck (PE alone: ~127us busy;
            # gather alone: 48 MB -> ~134us), so split: every 4th tile
            # gathers rolled from HBM, the rest use the PE.
            prev_up = None
            for k in range(NT):
                up = mp.tile([P, D], F32, tag="up")
                idx_col = idx0_i[:, 0:1] if k == 0 else idx_i[:, k : k + 1]
                nc.gpsimd.indirect_dma_start(
                    out=up[:], out_offset=None, in_=z[:],
                    in_offset=IndirectOffsetOnAxis(ap=idx_col, axis=0),
                )

                # t1 = p * up on ACT
                t1 = mp.tile([P, D], F32, tag="t1")
                nc.scalar.mul(out=t1[:], in_=up[:], mul=p_w[:, k : k + 1])

                o = mp.tile([P, D], BF16, tag="o")
                if use_gather_tiles and k >= NT - 2:
                    # tail tiles: HBM-gather `rolled` (HBM is idle by now) so
                    # the final stores don't wait on the PE matmul backlog
                    rolled = mp.tile([P, D], F32, tag="rolled")
                    nc.gpsimd.indirect_dma_start(
                        out=rolled[:], out_offset=None, in_=z[:],
                        in_offset=IndirectOffsetOnAxis(ap=idxp_i[:, k : k + 1], axis=0),
                    )
                    nc.vector.scalar_tensor_tensor(
                        out=o[:], in0=rolled[:], scalar=q_w[:, k : k + 1],
                        in1=t1[:],
                        op0=mybir.AluOpType.mult, op1=mybir.AluOpType.add,
                    )
                elif (k + 1) % SHIFT_STRIDE == 0 and prev_up is not None:
                    # rolled via partition-shifted SBUF->SBUF DMA (scalar ring)
                    rolled = mp.tile([P, D], F32, tag="rolled")
                    nc.scalar.dma_start(out=rolled[1:P, :], in_=up[0 : P - 1, :])
                    nc.scalar.dma_start(out=rolled[0:1, :], in_=prev_up[P - 1 : P, :])
                    nc.vector.scalar_tensor_tensor(
                        out=o[:], in0=rolled[:], scalar=q_w[:, k : k + 1],
                        in1=t1[:],
                        op0=mybir.AluOpType.mult, op1=mybir.AluOpType.add,
                    )
                else:
                    # rolled[i] = up[i-1] via PE (row 0 -> 0, fixed by epilogue)
                    rps = ppr.tile([P, D], F32, space="PSUM", tag="roll")
                    for h in range(2):
                        sl = slice(h * DH, (h + 1) * DH)
                        nc.tensor.matmul(out=rps[:, sl], lhsT=ishift[:], rhs=up[:, sl],
                                         start=True, stop=True, skip_group_check=True)
                    # o = (rolled * q) + t1 on DVE, one op across both banks
                    nc.vector.scalar_tensor_tensor(
                        out=o[:], in0=rps[:], scalar=q_w[:, k : k + 1],
                        in1=t1[:],
                        op0=mybir.AluOpType.mult, op1=mybir.AluOpType.add,
                    )

                nc.sync.dma_start(out=out[k * P : (k + 1) * P, :], in_=o[:])
                prev_up = up

                if k == 8:
                    # epilogue gathers + blend, issued mid-loop so they fill
                    # gather-stream slack instead of delaying tile 0 (gpsimd
                    # FIFO) or extending the tail; only the store is last.
                    upf = sp.tile([NCOL, D], F32)
                    nc.gpsimd.indirect_dma_start(
                        out=upf[:], out_offset=None, in_=z[:],
                        in_offset=IndirectOffsetOnAxis(ap=fidx_i[:, 0:1], axis=0),
                    )
                    rollf = sp.tile([NCOL, D], F32)
                    nc.gpsimd.indirect_dma_start(
                        out=rollf[:], out_offset=None, in_=z[:],
                        in_offset=IndirectOffsetOnAxis(ap=bidx_i[:, 0:1], axis=0),
                    )
                    t1b = sp.tile([NCOL, D], F32)
                    nc.scalar.mul(out=t1b[:], in_=upf[:], mul=pb_col[:])
                    ob = sp.tile([NCOL, D], BF16)
                    nc.vector.scalar_tensor_tensor(
                        out=ob[:], in0=rollf[:], scalar=qb_col[:], in1=t1b[:],
                        op0=mybir.AluOpType.mult, op1=mybir.AluOpType.add,
                    )

            # ---- epilogue store: rows t = 128j to their own tensor ---------
            nc.sync.dma_start(out=out2[:, :], in_=ob[:])

    # Run the Bacc lowering passes (register allocation, event-semaphore
    # splitting, ...) — run_bass_via_pjrt serializes nc.m as-is.
    nc.finalize()
    return nc


_NC_CACHE = None


def _get_nc() -> bass.Bass:
    global _NC_CACHE
    if _NC_CACHE is None:
        _NC_CACHE = build_bass()
    return _NC_CACHE


def make_in_maps(z: np.ndarray, p: np.ndarray, b: np.ndarray) -> list[dict]:
    return [
        {
            "z": np.ascontiguousarray(z[i], dtype=np.float32),
            "p": np.ascontiguousarray(p[i], dtype=np.float32),
            "b": np.ascontiguousarray(b[i], dtype=np.int32),
        }
        for i in range(B)
    ]


def kernel(z, p, b, original_len=None, **_unused) -> np.ndarray:
    z = np.asarray(z, dtype=np.float32)
    p = np.asarray(p, dtype=np.float32)
    b = np.asarray(b, dtype=np.int32)
    assert z.shape == (B, NCH, D) and p.shape == (B, T) and b.shape == (B, T)

    nc = _get_nc()
    res = run_bass_kernel_spmd(nc, make_in_maps(z, p, b), list(range(B)))
    outs = []
    for r in res.results:
        full = np.asarray(r["out"]).astype(np.float32)       # [T, D]
        rows0 = np.asarray(r["out2"]).astype(np.float32)     # [NT, D]
        full[0::P, :] = rows0                                # merge t = 128j rows
        outs.append(full)
    return np.stack(outs, axis=0)



# revision 31
# speedup vs baseline: 1.0005x; 1.0005x over previous
"""Trainium2 Bass kernel for nn_DechunkingLayer (ragged_sequence).

Reference semantics (per batch row):
    idx = clip(exclusive_cumsum(b), 0, NC - 1)          # [T]
    up[t]  = z[idx[t]]                                  # gather rows
    out[t] = p[t] * up[t] + (1 - p[t]) * up[t-1]        # EMA blend
    out[0] = up[0]

Sharding: pure data parallel over batch B=8 across the 8 NeuronCores
(one batch row per core). All work per row is independent.

Per-core plan (HBM traffic = 16 MB fp32 gather + 8 MB bf16 store):
  - exclusive cumsum of the 0/1 boundary flags computed on-device with a
    PE triangular-matmul scan in a [128, 32] "W layout" (partition = t % 128,
    column = t // 128) — exactly the layout the indirect-DMA gather wants
    its per-partition row indices in. Tile-0 indices take a short path so
    the first gather issues before the column-offset chain finishes.
  - rolled (up[t-1]) is the gathered tile shifted down one partition.
    Compute engines cannot read partition-shifted operands (windows must
    be 32-aligned), and SBUF->SBUF DMA shifts eat DMA-queue bandwidth
    (the binding resource: ~25 GB/s per queue x 16 queues), so the shift
    rides the PE as a shifted-identity fp32 matmul (bitwise exact).
    fp32 (4 cycles/row) is mandatory: fp32r/bf16 round the streamed data,
    which blows up relative error at cancellation points of the blend.
  - NO PE warmup: the HAM clock gate throttles the PE until ~4us of
    cumulative busy, but warmup matmuls delay the cumsum chain / first
    gather more than the brief half-clock ramp costs.
  - the final store is bf16: rounding the FINAL value keeps max relative
    error <= 2^-9 even under cancellation (rounding any blend INPUT would
    not); host upcasts to fp32. Halves store traffic.
  - per-tile rows t=128k blend against the previous tile's last row; those
    32 rows are redone exactly in a small epilogue pass (2 gathers of 32
    rows + blend) stored to a SEPARATE output tensor (no write-after-write
    wait on the 32 main stores, which would serialize the tail); the host
    merges the rows back (pure assembly, no arithmetic).
  - out[0] = up[0] exactly via forcing p[0] = 1 (q[0] = 0).
"""

import numpy as np

import concourse.bacc as bacc
import concourse.bass as bass
import concourse.mybir as mybir
import concourse.tile as tile
from concourse.bass import IndirectOffsetOnAxis
from concourse.bass_utils import run_bass_kernel_spmd
from concourse.masks import make_identity, make_upper_triangular

# Problem shape (hardcoded per harness contract).
B = 8          # batch rows == number of cores
T = 4096       # timesteps per row
NCH = 2048     # number of chunks (z rows)
D = 1024       # d_model
P = 128        # SBUF partitions
NT = T // P    # 32 tiles per core
NCOL = T // P  # 32 columns in the W layout
DH = D // 2    # matmul free-dim max for fp32 is 512

F32 = mybir.dt.float32
F32R = mybir.dt.float32r
BF16 = mybir.dt.bfloat16
I32 = mybir.dt.int32

# Measured dead ends, kept disabled: a second HBM gather for `rolled`
# (GATHER_STRIDE) loses to the PE shift; a partition-shifted SBUF->SBUF
# DMA (SHIFT_STRIDE) eats DMA-queue bytes, the binding resource.
GATHER_STRIDE = 1000  # > NT: disabled
SHIFT_STRIDE = 1000   # > NT: disabled
WARMUP_MM = 5         # unused (warmup removed; loop self-warms the PE)


def build_bass() -> bass.Bass:
    # Bacc (not raw Bass): its finalize() runs generate_event_semaphores,
    # which splits multi-sem waits to satisfy TRN2's one-wait-per-instruction
    # ISA constraint.
    nc = bacc.Bacc()

    z = nc.dram_tensor("z", [NCH, D], F32, kind="ExternalInput")
    p = nc.dram_tensor("p", [T], F32, kind="ExternalInput")
    b = nc.dram_tensor("b", [T], I32, kind="ExternalInput")
    out = nc.dram_tensor("out", [T, D], BF16, kind="ExternalOutput")
    # rows t = 128j are redone exactly; they go to a SEPARATE tensor so
    # their store has no write-after-write dependency on the 32 main
    # stores (which would serialize the tail); the host merges them.
    out2 = nc.dram_tensor("out2", [NCOL, D], BF16, kind="ExternalOutput")

    with tile.TileContext(nc) as tc:
        with (
            tc.tile_pool(name="setup", bufs=1) as sp,
            tc.tile_pool(name="psmall", bufs=2, space="PSUM") as pps,
            tc.tile_pool(name="proll", bufs=3, space="PSUM") as ppr,
            tc.tile_pool(name="main", bufs=14) as mp,
        ):
            # ---- constants -------------------------------------------------
            # affine_select only exists on gpsimd; PE Matmult has a single
            # sync-wait slot, so launder every matmul operand through DVE so
            # all matmul waits collapse onto one DVE semaphore.
            tri_g = sp.tile([P, P], F32)     # tri[k, i] = 1 iff i > k
            make_upper_triangular(nc, tri_g[:], val=1.0, diag=False)
            tri = sp.tile([P, P], F32)
            nc.vector.tensor_copy(out=tri[:], in_=tri_g[:])

            ident_g = sp.tile([NCOL, NCOL], F32)
            make_identity(nc, ident_g[:])
            ident = sp.tile([NCOL, NCOL], F32)
            nc.vector.tensor_copy(out=ident[:], in_=ident_g[:])

            tri32_g = sp.tile([NCOL, NCOL], F32)  # [k, j] = 1 iff j > k
            make_upper_triangular(nc, tri32_g[:], val=1.0, diag=False)
            tri32 = sp.tile([NCOL, NCOL], F32)
            nc.vector.tensor_copy(out=tri32[:], in_=tri32_g[:])

            # shifted identity: S[k, i] = 1 iff i == k + 1  ->  (S^T @ x)[i] = x[i-1]
            ish_g = sp.tile([P, P], F32)
            nc.gpsimd.memset(ish_g[:], 0.0)
            nc.gpsimd.affine_select(
                out=ish_g[:], in_=ish_g[:],
                compare_op=mybir.AluOpType.not_equal, fill=1.0,
                base=1, pattern=[[-1, P]], channel_multiplier=1,
            )
            ishift = sp.tile([P, P], F32)
            nc.vector.tensor_copy(out=ishift[:], in_=ish_g[:])

            ones_row = sp.tile([1, P], F32)  # lhsT for partition-broadcast
            nc.vector.memset(ones_row[:], 1.0)
            ones_col = sp.tile([P, 1], F32)  # lhsT for column sums
            nc.vector.memset(ones_col[:], 1.0)


            # ---- load b and p in natural [32, 128] layout ------------------
            b2d = b[:].rearrange("(j c) -> j c", c=P)          # [32, 128] DRAM view
            p2d = p[:].rearrange("(j c) -> j c", c=P)

            b_nat_i = sp.tile([NCOL, P], I32)
            nc.sync.dma_start(out=b_nat_i[:], in_=b2d)
            p_nat = sp.tile([NCOL, P], F32)
            nc.sync.dma_start(out=p_nat[:], in_=p2d)

            b_nat = sp.tile([NCOL, P], F32)
            nc.vector.tensor_copy(out=b_nat[:], in_=b_nat_i[:])

            # b_shifted[t] = b[t-1] (0 at t=0) for idx_prev of the gather-tiles
            use_gather_tiles = False  # PE shift is fast; no tail trim needed
            if use_gather_tiles:
                bp_nat_i = sp.tile([NCOL, P], I32)
                nc.vector.memset(bp_nat_i[0:1, 0:1], 0)
                nc.sync.dma_start(out=bp_nat_i[:, 1:P], in_=b2d[:, 0 : P - 1])
                nc.sync.dma_start(
                    out=bp_nat_i[1:NCOL, 0:1], in_=b2d[0 : NCOL - 1, P - 1 : P]
                )
                bp_nat = sp.tile([NCOL, P], F32)
                nc.vector.tensor_copy(out=bp_nat[:], in_=bp_nat_i[:])

            # ---- PE transpose to W layout [128, 32]: (p, j) = t = 128j + p --
            bw_ps = pps.tile([P, NCOL], F32, space="PSUM", tag="small_ps")
            nc.tensor.transpose(out=bw_ps[:], in_=b_nat[:], identity=ident[:])
            b_w = sp.tile([P, NCOL], F32)
            nc.vector.tensor_copy(out=b_w[:], in_=bw_ps[:])

            # tile-0 indices on a short path: colofs[0] = 0, so column 0
            # needs only the partition scan — the first gather can issue
            # before the column-offset chain finishes.
            s0_ps = pps.tile([P, 1], F32, space="PSUM", tag="small_ps")
            nc.tensor.matmul(out=s0_ps[:], lhsT=tri[:], rhs=b_w[:, 0:1],
                             start=True, stop=True)
            idx0_f = sp.tile([P, 1], F32)
            nc.vector.tensor_scalar_min(out=idx0_f[:], in0=s0_ps[:],
                                        scalar1=float(NCH - 1))
            idx0_i = sp.tile([P, 1], I32)
            nc.vector.tensor_copy(out=idx0_i[:], in_=idx0_f[:])

            if use_gather_tiles:
                bpw_ps = pps.tile([P, NCOL], F32, space="PSUM", tag="small_ps")
                nc.tensor.transpose(out=bpw_ps[:], in_=bp_nat[:], identity=ident[:])
                bp_w = sp.tile([P, NCOL], F32)
                nc.vector.tensor_copy(out=bp_w[:], in_=bpw_ps[:])

            pw_ps = pps.tile([P, NCOL], F32, space="PSUM", tag="small_ps")
            nc.tensor.transpose(out=pw_ps[:], in_=p_nat[:], identity=ident[:])
            p_w = sp.tile([P, NCOL], F32)
            nc.vector.tensor_copy(out=p_w[:], in_=pw_ps[:])
            # out[0] = up[0] exactly: force p[0] = 1 so the blend is 1*up + 0*rolled
            nc.vector.memset(p_w[0:1, 0:1], 1.0)
            q_w = sp.tile([P, NCOL], F32)  # q = 1 - p
            nc.scalar.activation(
                out=q_w[:], in_=p_w[:],
                func=mybir.ActivationFunctionType.Copy, bias=1.0, scale=-1.0,
            )

            # ---- column offsets via two PE matmuls -------------------------
            # tot_col[j'] = sum_k b_w[k, j'] as a column, then
            # colofs[0, j] = sum_{j'<j} tot[j'] via the strict triangular.
            totc_ps = pps.tile([NCOL, 1], F32, space="PSUM", tag="small_ps")
            nc.tensor.matmul(out=totc_ps[:], lhsT=b_w[:], rhs=ones_col[:],
                             start=True, stop=True)
            tot_col = sp.tile([NCOL, 1], F32)
            nc.vector.tensor_copy(out=tot_col[:], in_=totc_ps[:])
            cofs_ps = pps.tile([1, NCOL], F32, space="PSUM", tag="small_ps")
            nc.tensor.matmul(out=cofs_ps[:], lhsT=tot_col[:], rhs=tri32[:],
                             start=True, stop=True)
            colofs = sp.tile([1, NCOL], F32)
            nc.vector.tensor_copy(out=colofs[:], in_=cofs_ps[:])

            # ---- full exclusive cumsum s[t] in W layout --------------------
            # s_ps[i, j] = sum_{k<i} b_w[k, j]  +  colofs[j]
            s_ps = pps.tile([P, NCOL], F32, space="PSUM", tag="small_ps")
            nc.tensor.matmul(out=s_ps[:], lhsT=tri[:], rhs=b_w[:],
                             start=True, stop=False)
            nc.tensor.matmul(out=s_ps[:], lhsT=ones_row[:], rhs=colofs[:],
                             start=False, stop=True)

            # ---- gather indices: idx = min(s, NCH-1) -----------------------
            idx_f = sp.tile([P, NCOL], F32)
            nc.vector.tensor_scalar_min(out=idx_f[:], in0=s_ps[:], scalar1=float(NCH - 1))
            idx_i = sp.tile([P, NCOL], I32)
            nc.vector.tensor_copy(out=idx_i[:], in_=idx_f[:])

            # idx_prev = min(s - b_shifted, NCH-1)  (s[t] - b[t-1] = s[t-1])
            if use_gather_tiles:
                sprev_f = sp.tile([P, NCOL], F32)
                nc.vector.tensor_sub(out=sprev_f[:], in0=s_ps[:], in1=bp_w[:])
                idxp_f = sp.tile([P, NCOL], F32)
                nc.vector.tensor_scalar_min(
                    out=idxp_f[:], in0=sprev_f[:], scalar1=float(NCH - 1)
                )
                idxp_i = sp.tile([P, NCOL], I32)
                nc.vector.tensor_copy(out=idxp_i[:], in_=idxp_f[:])

            # ---- epilogue vectors for rows t = 128j ------------------------
            # bprev_row[j] = idx[128j - 1] (0 for j=0, harmless: q[0]=0).
            # Row 127 of idx_f is not a legal compute-engine base, so extract
            # it with a tiny SBUF->SBUF DMA, then rotate rows into columns
            # with [1,32]-lhsT matmuls against a single 1.0.
            bprev_row = sp.tile([1, NCOL], F32)
            nc.vector.memset(bprev_row[:], 0.0)
            nc.sync.dma_start(
                out=bprev_row[0:1, 1:NCOL], in_=idx_f[P - 1 : P, 0 : NCOL - 1]
            )

            cols_ps = pps.tile([NCOL, 4], F32, space="PSUM", tag="small_ps")
            for ci, row in enumerate([bprev_row, idx_f, p_w, q_w]):
                nc.tensor.matmul(
                    out=cols_ps[:, ci : ci + 1],
                    lhsT=row[0:1, 0:NCOL],
                    rhs=ones_row[0:1, 0:1],
                    start=True, stop=True,
                )
            bidx_i = sp.tile([NCOL, 1], I32)
            nc.vector.tensor_copy(out=bidx_i[:], in_=cols_ps[:, 0:1])
            fidx_i = sp.tile([NCOL, 1], I32)
            nc.vector.tensor_copy(out=fidx_i[:], in_=cols_ps[:, 1:2])
            pb_col = sp.tile([NCOL, 1], F32)
            nc.vector.tensor_copy(out=pb_col[:], in_=cols_ps[:, 2:3])
            qb_col = sp.tile([NCOL, 1], F32)
            nc.vector.tensor_copy(out=qb_col[:], in_=cols_ps[:, 3:4])

            # ---- main loop: gather, roll, blend, store ---------------------
            # The roll (rolled[i] = up[i-1]) costs either PE time (shifted-
            # identity matmul, exact; fp32 runs HI/LO = 2 passes) or HBM
            # bandwidth (a second gather). Neither engine can absorb all 32
            # tiles without becoming the bottleneck (PE alone: ~127us busy;
            # gather alone: 48 MB -> ~134us), so split: every 4th tile
            # gathers rolled from HBM, the rest use the PE.
            prev_up = None
            for k in range(NT):
                up = mp.tile([P, D], F32, tag="up")
                idx_col = idx0_i[:, 0:1] if k == 0 else idx_i[:, k : k + 1]
                nc.gpsimd.indirect_dma_start(
                    out=up[:], out_offset=None, in_=z[:],
                    in_offset=IndirectOffsetOnAxis(ap=idx_col, axis=0),
                )

                # t1 = p * up on ACT
                t1 = mp.tile([P, D], F32, tag="t1")
                nc.scalar.mul(out=t1[:], in_=up[:], mul=p_w[:, k : k + 1])

                o = mp.tile([P, D], BF16, tag="o")
                if use_gather_tiles and k >= NT - 2:
                    # tail tiles: HBM-gather `rolled` (HBM is idle by now) so
                    # the final stores don't wait on the PE matmul backlog
                    rolled = mp.tile([P, D], F32, tag="rolled")
                    nc.gpsimd.indirect_dma_start(
                        out=rolled[:], out_offset=None, in_=z[:],
                        in_offset=IndirectOffsetOnAxis(ap=idxp_i[:, k : k + 1], axis=0),
                    )
                    nc.vector.scalar_tensor_tensor(
                        out=o[:], in0=rolled[:], scalar=q_w[:, k : k + 1],
                        in1=t1[:],
                        op0=mybir.AluOpType.mult, op1=mybir.AluOpType.add,
                    )
                elif (k + 1) % SHIFT_STRIDE == 0 and prev_up is not None:
                    # rolled via partition-shifted SBUF->SBUF DMA (scalar ring)
                    rolled = mp.tile([P, D], F32, tag="rolled")
                    nc.scalar.dma_start(out=rolled[1:P, :], in_=up[0 : P - 1, :])
                    nc.scalar.dma_start(out=rolled[0:1, :], in_=prev_up[P - 1 : P, :])
                    nc.vector.scalar_tensor_tensor(
                        out=o[:], in0=rolled[:], scalar=q_w[:, k : k + 1],
                        in1=t1[:],
                        op0=mybir.AluOpType.mult, op1=mybir.AluOpType.add,
                    )
                else:
                    # rolled[i] = up[i-1] via PE (row 0 -> 0, fixed by epilogue)
                    rps = ppr.tile([P, D], F32, space="PSUM", tag="roll")
                    for h in range(2):
                        sl = slice(h * DH, (h + 1) * DH)
                        nc.tensor.matmul(out=rps[:, sl], lhsT=ishift[:], rhs=up[:, sl],
                                         start=True, stop=True, skip_group_check=True)
                    # o = (rolled * q) + t1 on DVE, one op across both banks
                    nc.vector.scalar_tensor_tensor(
                        out=o[:], in0=rps[:], scalar=q_w[:, k : k + 1],
                        in1=t1[:],
                        op0=mybir.AluOpType.mult, op1=mybir.AluOpType.add,
                    )

                nc.sync.dma_start(out=out[k * P : (k + 1) * P, :], in_=o[:])
                prev_up = up

                if k == 8:
                    # epilogue gathers + blend, issued mid-loop so they fill
                    # gather-stream slack instead of delaying tile 0 (gpsimd
                    # FIFO) or extending the tail; only the store is last.
                    upf = sp.tile([NCOL, D], F32)
                    nc.gpsimd.indirect_dma_start(
                        out=upf[:], out_offset=None, in_=z[:],
                        in_offset=IndirectOffsetOnAxis(ap=fidx_i[:, 0:1], axis=0),
                    )
                    rollf = sp.tile([NCOL, D], F32)
                    nc.gpsimd.indirect_dma_start(
                        out=rollf[:], out_offset=None, in_=z[:],
                        in_offset=IndirectOffsetOnAxis(ap=bidx_i[:, 0:1], axis=0),
                    )
                    t1b = sp.tile([NCOL, D], F32)
                    nc.scalar.mul(out=t1b[:], in_=upf[:], mul=pb_col[:])
                    ob = sp.tile([NCOL, D], BF16)
                    nc.vector.scalar_tensor_tensor(
                        out=ob[:], in0=rollf[:], scalar=qb_col[:], in1=t1b[:],
                        op0=mybir.AluOpType.mult, op1=mybir.AluOpType.add,
                    )

            # ---- epilogue store: rows t = 128j to their own tensor ---------
            nc.sync.dma_start(out=out2[:, :], in_=ob[:])

    # Run the Bacc lowering passes (register allocation, event-semaphore
    # splitting, ...) — run_bass_via_pjrt serializes nc.m as-is.
    nc.finalize()
    return nc


_NC_CACHE = None


def _get_nc() -> bass.Bass:
    global _NC_CACHE
    if _NC_CACHE is None:
        _NC_CACHE = build_bass()
    return _NC_CACHE


def make_in_maps(z: np.ndarray, p: np.ndarray, b: np.ndarray) -> list[dict]:
    return [
        {
            "z": np.ascontiguousarray(z[i], dtype=np.float32),
            "p": np.ascontiguousarray(p[i], dtype=np.float32),
            "b": np.ascontiguousarray(b[i], dtype=np.int32),
        }
        for i in range(B)
    ]


def kernel(z, p, b, original_len=None, **_unused) -> np.ndarray:
    z = np.asarray(z, dtype=np.float32)
    p = np.asarray(p, dtype=np.float32)
    b = np.asarray(b, dtype=np.int32)
    assert z.shape == (B, NCH, D) and p.shape == (B, T) and b.shape == (B, T)

    nc = _get_nc()
    res = run_bass_kernel_spmd(nc, make_in_maps(z, p, b), list(range(B)))
    outs = []
    for r in res.results:
        full = np.asarray(r["out"]).astype(np.float32)       # [T, D]
        rows0 = np.asarray(r["out2"]).astype(np.float32)     # [NT, D]
        full[0::P, :] = rows0                                # merge t = 128j rows
        outs.append(full)
    return np.stack(outs, axis=0)



# revision 32
# speedup vs baseline: 1.0131x; 1.0126x over previous
"""Trainium2 Bass kernel for nn_DechunkingLayer (ragged_sequence).

Reference semantics (per batch row):
    idx = clip(exclusive_cumsum(b), 0, NC - 1)          # [T]
    up[t]  = z[idx[t]]                                  # gather rows
    out[t] = p[t] * up[t] + (1 - p[t]) * up[t-1]        # EMA blend
    out[0] = up[0]

Sharding: pure data parallel over batch B=8 across the 8 NeuronCores
(one batch row per core). All work per row is independent.

Per-core plan (HBM traffic = 16 MB fp32 gather + 8 MB bf16 store):
  - exclusive cumsum of the 0/1 boundary flags computed on-device with a
    PE triangular-matmul scan in a [128, 32] "W layout" (partition = t % 128,
    column = t // 128) — exactly the layout the indirect-DMA gather wants
    its per-partition row indices in. Tile-0 indices take a short path so
    the first gather issues before the column-offset chain finishes.
  - rolled (up[t-1]) is the gathered tile shifted down one partition.
    Compute engines cannot read partition-shifted operands (windows must
    be 32-aligned), and SBUF->SBUF DMA shifts eat DMA-queue bandwidth
    (the binding resource: ~25 GB/s per queue x 16 queues), so the shift
    rides the PE as a shifted-identity fp32 matmul (bitwise exact).
    fp32 (4 cycles/row) is mandatory: fp32r/bf16 round the streamed data,
    which blows up relative error at cancellation points of the blend.
  - NO PE warmup: the HAM clock gate throttles the PE until ~4us of
    cumulative busy, but warmup matmuls delay the cumsum chain / first
    gather more than the brief half-clock ramp costs.
  - the final store is bf16: rounding the FINAL value keeps max relative
    error <= 2^-9 even under cancellation (rounding any blend INPUT would
    not); host upcasts to fp32. Halves store traffic.
  - per-tile rows t=128k blend against the previous tile's last row; those
    32 rows are redone exactly in a small epilogue pass (2 gathers of 32
    rows + blend) stored to a SEPARATE output tensor (no write-after-write
    wait on the 32 main stores, which would serialize the tail); the host
    merges the rows back (pure assembly, no arithmetic).
  - out[0] = up[0] exactly via forcing p[0] = 1 (q[0] = 0).
"""

import numpy as np

import concourse.bacc as bacc
import concourse.bass as bass
import concourse.mybir as mybir
import concourse.tile as tile
from concourse.bass import IndirectOffsetOnAxis
from concourse.bass_utils import run_bass_kernel_spmd
from concourse.masks import make_identity, make_upper_triangular

# Problem shape (hardcoded per harness contract).
B = 8          # batch rows == number of cores
T = 4096       # timesteps per row
NCH = 2048     # number of chunks (z rows)
D = 1024       # d_model
P = 128        # SBUF partitions
NT = T // P    # 32 tiles per core
NCOL = T // P  # 32 columns in the W layout
DH = D // 2    # matmul free-dim max for fp32 is 512

F32 = mybir.dt.float32
F32R = mybir.dt.float32r
BF16 = mybir.dt.bfloat16
I32 = mybir.dt.int32

# Measured dead ends, kept disabled: a second HBM gather for `rolled`
# (GATHER_STRIDE) loses to the PE shift; a partition-shifted SBUF->SBUF
# DMA (SHIFT_STRIDE) eats DMA-queue bytes, the binding resource.
GATHER_STRIDE = 1000  # > NT: disabled
SHIFT_STRIDE = 1000   # > NT: disabled
WARMUP_MM = 5         # unused (warmup removed; loop self-warms the PE)


def build_bass() -> bass.Bass:
    # Bacc (not raw Bass): its finalize() runs generate_event_semaphores,
    # which splits multi-sem waits to satisfy TRN2's one-wait-per-instruction
    # ISA constraint.
    nc = bacc.Bacc()

    z = nc.dram_tensor("z", [NCH, D], F32, kind="ExternalInput")
    p = nc.dram_tensor("p", [T], F32, kind="ExternalInput")
    b = nc.dram_tensor("b", [T], I32, kind="ExternalInput")
    out = nc.dram_tensor("out", [T, D], BF16, kind="ExternalOutput")
    # rows t = 128j are redone exactly; they go to a SEPARATE tensor so
    # their store has no write-after-write dependency on the 32 main
    # stores (which would serialize the tail); the host merges them.
    out2 = nc.dram_tensor("out2", [NCOL, D], BF16, kind="ExternalOutput")

    with tile.TileContext(nc) as tc:
        with (
            tc.tile_pool(name="setup", bufs=1) as sp,
            tc.tile_pool(name="psmall", bufs=2, space="PSUM") as pps,
            tc.tile_pool(name="proll", bufs=3, space="PSUM") as ppr,
            tc.tile_pool(name="main", bufs=10) as mp,
        ):
            # ---- constants -------------------------------------------------
            # affine_select only exists on gpsimd; PE Matmult has a single
            # sync-wait slot, so launder every matmul operand through DVE so
            # all matmul waits collapse onto one DVE semaphore.
            tri_g = sp.tile([P, P], F32)     # tri[k, i] = 1 iff i > k
            make_upper_triangular(nc, tri_g[:], val=1.0, diag=False)
            tri = sp.tile([P, P], F32)
            nc.vector.tensor_copy(out=tri[:], in_=tri_g[:])

            ident_g = sp.tile([NCOL, NCOL], F32)
            make_identity(nc, ident_g[:])
            ident = sp.tile([NCOL, NCOL], F32)
            nc.vector.tensor_copy(out=ident[:], in_=ident_g[:])

            tri32_g = sp.tile([NCOL, NCOL], F32)  # [k, j] = 1 iff j > k
            make_upper_triangular(nc, tri32_g[:], val=1.0, diag=False)
            tri32 = sp.tile([NCOL, NCOL], F32)
            nc.vector.tensor_copy(out=tri32[:], in_=tri32_g[:])

            # shifted identity: S[k, i] = 1 iff i == k + 1  ->  (S^T @ x)[i] = x[i-1]
            ish_g = sp.tile([P, P], F32)
            nc.gpsimd.memset(ish_g[:], 0.0)
            nc.gpsimd.affine_select(
                out=ish_g[:], in_=ish_g[:],
                compare_op=mybir.AluOpType.not_equal, fill=1.0,
                base=1, pattern=[[-1, P]], channel_multiplier=1,
            )
            ishift = sp.tile([P, P], F32)
            nc.vector.tensor_copy(out=ishift[:], in_=ish_g[:])

            ones_row = sp.tile([1, P], F32)  # lhsT for partition-broadcast
            nc.vector.memset(ones_row[:], 1.0)
            ones_col = sp.tile([P, 1], F32)  # lhsT for column sums
            nc.vector.memset(ones_col[:], 1.0)


            # ---- load b and p in natural [32, 128] layout ------------------
            b2d = b[:].rearrange("(j c) -> j c", c=P)          # [32, 128] DRAM view
            p2d = p[:].rearrange("(j c) -> j c", c=P)

            b_nat_i = sp.tile([NCOL, P], I32)
            nc.sync.dma_start(out=b_nat_i[:], in_=b2d)
            p_nat = sp.tile([NCOL, P], F32)
            nc.sync.dma_start(out=p_nat[:], in_=p2d)

            b_nat = sp.tile([NCOL, P], F32)
            nc.vector.tensor_copy(out=b_nat[:], in_=b_nat_i[:])

            # b_shifted[t] = b[t-1] (0 at t=0) for idx_prev of the gather-tiles
            use_gather_tiles = False  # PE shift is fast; no tail trim needed
            if use_gather_tiles:
                bp_nat_i = sp.tile([NCOL, P], I32)
                nc.vector.memset(bp_nat_i[0:1, 0:1], 0)
                nc.sync.dma_start(out=bp_nat_i[:, 1:P], in_=b2d[:, 0 : P - 1])
                nc.sync.dma_start(
                    out=bp_nat_i[1:NCOL, 0:1], in_=b2d[0 : NCOL - 1, P - 1 : P]
                )
                bp_nat = sp.tile([NCOL, P], F32)
                nc.vector.tensor_copy(out=bp_nat[:], in_=bp_nat_i[:])

            # ---- PE transpose to W layout [128, 32]: (p, j) = t = 128j + p --
            bw_ps = pps.tile([P, NCOL], F32, space="PSUM", tag="small_ps")
            nc.tensor.transpose(out=bw_ps[:], in_=b_nat[:], identity=ident[:])
            b_w = sp.tile([P, NCOL], F32)
            nc.vector.tensor_copy(out=b_w[:], in_=bw_ps[:])

            # tile-0 indices on a short path: colofs[0] = 0, so column 0
            # needs only the partition scan — the first gather can issue
            # before the column-offset chain finishes.
            s0_ps = pps.tile([P, 1], F32, space="PSUM", tag="small_ps")
            nc.tensor.matmul(out=s0_ps[:], lhsT=tri[:], rhs=b_w[:, 0:1],
                             start=True, stop=True)
            idx0_f = sp.tile([P, 1], F32)
            nc.vector.tensor_scalar_min(out=idx0_f[:], in0=s0_ps[:],
                                        scalar1=float(NCH - 1))
            idx0_i = sp.tile([P, 1], I32)
            nc.vector.tensor_copy(out=idx0_i[:], in_=idx0_f[:])

            if use_gather_tiles:
                bpw_ps = pps.tile([P, NCOL], F32, space="PSUM", tag="small_ps")
                nc.tensor.transpose(out=bpw_ps[:], in_=bp_nat[:], identity=ident[:])
                bp_w = sp.tile([P, NCOL], F32)
                nc.vector.tensor_copy(out=bp_w[:], in_=bpw_ps[:])

            pw_ps = pps.tile([P, NCOL], F32, space="PSUM", tag="small_ps")
            nc.tensor.transpose(out=pw_ps[:], in_=p_nat[:], identity=ident[:])
            p_w = sp.tile([P, NCOL], F32)
            nc.vector.tensor_copy(out=p_w[:], in_=pw_ps[:])
            # out[0] = up[0] exactly: force p[0] = 1 so the blend is 1*up + 0*rolled
            nc.vector.memset(p_w[0:1, 0:1], 1.0)
            q_w = sp.tile([P, NCOL], F32)  # q = 1 - p
            nc.scalar.activation(
                out=q_w[:], in_=p_w[:],
                func=mybir.ActivationFunctionType.Copy, bias=1.0, scale=-1.0,
            )

            # ---- column offsets via two PE matmuls -------------------------
            # tot_col[j'] = sum_k b_w[k, j'] as a column, then
            # colofs[0, j] = sum_{j'<j} tot[j'] via the strict triangular.
            totc_ps = pps.tile([NCOL, 1], F32, space="PSUM", tag="small_ps")
            nc.tensor.matmul(out=totc_ps[:], lhsT=b_w[:], rhs=ones_col[:],
                             start=True, stop=True)
            tot_col = sp.tile([NCOL, 1], F32)
            nc.vector.tensor_copy(out=tot_col[:], in_=totc_ps[:])
            cofs_ps = pps.tile([1, NCOL], F32, space="PSUM", tag="small_ps")
            nc.tensor.matmul(out=cofs_ps[:], lhsT=tot_col[:], rhs=tri32[:],
                             start=True, stop=True)
            colofs = sp.tile([1, NCOL], F32)
            nc.vector.tensor_copy(out=colofs[:], in_=cofs_ps[:])

            # ---- full exclusive cumsum s[t] in W layout --------------------
            # s_ps[i, j] = sum_{k<i} b_w[k, j]  +  colofs[j]
            s_ps = pps.tile([P, NCOL], F32, space="PSUM", tag="small_ps")
            nc.tensor.matmul(out=s_ps[:], lhsT=tri[:], rhs=b_w[:],
                             start=True, stop=False)
            nc.tensor.matmul(out=s_ps[:], lhsT=ones_row[:], rhs=colofs[:],
                             start=False, stop=True)

            # ---- gather indices: idx = min(s, NCH-1) -----------------------
            idx_f = sp.tile([P, NCOL], F32)
            nc.vector.tensor_scalar_min(out=idx_f[:], in0=s_ps[:], scalar1=float(NCH - 1))
            idx_i = sp.tile([P, NCOL], I32)
            nc.vector.tensor_copy(out=idx_i[:], in_=idx_f[:])

            # idx_prev = min(s - b_shifted, NCH-1)  (s[t] - b[t-1] = s[t-1])
            if use_gather_tiles:
                sprev_f = sp.tile([P, NCOL], F32)
                nc.vector.tensor_sub(out=sprev_f[:], in0=s_ps[:], in1=bp_w[:])
                idxp_f = sp.tile([P, NCOL], F32)
                nc.vector.tensor_scalar_min(
                    out=idxp_f[:], in0=sprev_f[:], scalar1=float(NCH - 1)
                )
                idxp_i = sp.tile([P, NCOL], I32)
                nc.vector.tensor_copy(out=idxp_i[:], in_=idxp_f[:])

            # ---- epilogue vectors for rows t = 128j ------------------------
            # bprev_row[j] = idx[128j - 1] (0 for j=0, harmless: q[0]=0).
            # Row 127 of idx_f is not a legal compute-engine base, so extract
            # it with a tiny SBUF->SBUF DMA, then rotate rows into columns
            # with [1,32]-lhsT matmuls against a single 1.0.
            bprev_row = sp.tile([1, NCOL], F32)
            nc.vector.memset(bprev_row[:], 0.0)
            nc.sync.dma_start(
                out=bprev_row[0:1, 1:NCOL], in_=idx_f[P - 1 : P, 0 : NCOL - 1]
            )

            cols_ps = pps.tile([NCOL, 4], F32, space="PSUM", tag="small_ps")
            for ci, row in enumerate([bprev_row, idx_f, p_w, q_w]):
                nc.tensor.matmul(
                    out=cols_ps[:, ci : ci + 1],
                    lhsT=row[0:1, 0:NCOL],
                    rhs=ones_row[0:1, 0:1],
                    start=True, stop=True,
                )
            bidx_i = sp.tile([NCOL, 1], I32)
            nc.vector.tensor_copy(out=bidx_i[:], in_=cols_ps[:, 0:1])
            fidx_i = sp.tile([NCOL, 1], I32)
            nc.vector.tensor_copy(out=fidx_i[:], in_=cols_ps[:, 1:2])
            pb_col = sp.tile([NCOL, 1], F32)
            nc.vector.tensor_copy(out=pb_col[:], in_=cols_ps[:, 2:3])
            qb_col = sp.tile([NCOL, 1], F32)
            nc.vector.tensor_copy(out=qb_col[:], in_=cols_ps[:, 3:4])

            # ---- main loop: gather, roll, blend, store ---------------------
            # The roll (rolled[i] = up[i-1]) costs either PE time (shifted-
            # identity matmul, exact; fp32 runs HI/LO = 2 passes) or HBM
            # bandwidth (a second gather). Neither engine can absorb all 32
            # tiles without becoming the bottleneck (PE alone: ~127us busy;
            # gather alone: 48 MB -> ~134us), so split: every 4th tile
            # gathers rolled from HBM, the rest use the PE.
            prev_up = None
            for k in range(NT):
                up = mp.tile([P, D], F32, tag="up")
                idx_col = idx0_i[:, 0:1] if k == 0 else idx_i[:, k : k + 1]
                nc.gpsimd.indirect_dma_start(
                    out=up[:], out_offset=None, in_=z[:],
                    in_offset=IndirectOffsetOnAxis(ap=idx_col, axis=0),
                )

                # t1 = p * up on ACT
                t1 = mp.tile([P, D], F32, tag="t1")
                nc.scalar.mul(out=t1[:], in_=up[:], mul=p_w[:, k : k + 1])

                o = mp.tile([P, D], BF16, tag="o")
                if use_gather_tiles and k >= NT - 2:
                    # tail tiles: HBM-gather `rolled` (HBM is idle by now) so
                    # the final stores don't wait on the PE matmul backlog
                    rolled = mp.tile([P, D], F32, tag="rolled")
                    nc.gpsimd.indirect_dma_start(
                        out=rolled[:], out_offset=None, in_=z[:],
                        in_offset=IndirectOffsetOnAxis(ap=idxp_i[:, k : k + 1], axis=0),
                    )
                    nc.vector.scalar_tensor_tensor(
                        out=o[:], in0=rolled[:], scalar=q_w[:, k : k + 1],
                        in1=t1[:],
                        op0=mybir.AluOpType.mult, op1=mybir.AluOpType.add,
                    )
                elif (k + 1) % SHIFT_STRIDE == 0 and prev_up is not None:
                    # rolled via partition-shifted SBUF->SBUF DMA (scalar ring)
                    rolled = mp.tile([P, D], F32, tag="rolled")
                    nc.scalar.dma_start(out=rolled[1:P, :], in_=up[0 : P - 1, :])
                    nc.scalar.dma_start(out=rolled[0:1, :], in_=prev_up[P - 1 : P, :])
                    nc.vector.scalar_tensor_tensor(
                        out=o[:], in0=rolled[:], scalar=q_w[:, k : k + 1],
                        in1=t1[:],
                        op0=mybir.AluOpType.mult, op1=mybir.AluOpType.add,
                    )
                else:
                    # rolled[i] = up[i-1] via PE (row 0 -> 0, fixed by epilogue)
                    rps = ppr.tile([P, D], F32, space="PSUM", tag="roll")
                    for h in range(2):
                        sl = slice(h * DH, (h + 1) * DH)
                        nc.tensor.matmul(out=rps[:, sl], lhsT=ishift[:], rhs=up[:, sl],
                                         start=True, stop=True, skip_group_check=True)
                    # o = (rolled * q) + t1 on DVE, one op across both banks
                    nc.vector.scalar_tensor_tensor(
                        out=o[:], in0=rps[:], scalar=q_w[:, k : k + 1],
                        in1=t1[:],
                        op0=mybir.AluOpType.mult, op1=mybir.AluOpType.add,
                    )

                nc.sync.dma_start(out=out[k * P : (k + 1) * P, :], in_=o[:])
                prev_up = up

                if k == 8:
                    # epilogue gathers + blend, issued mid-loop so they fill
                    # gather-stream slack instead of delaying tile 0 (gpsimd
                    # FIFO) or extending the tail; only the store is last.
                    upf = sp.tile([NCOL, D], F32)
                    nc.gpsimd.indirect_dma_start(
                        out=upf[:], out_offset=None, in_=z[:],
                        in_offset=IndirectOffsetOnAxis(ap=fidx_i[:, 0:1], axis=0),
                    )
                    rollf = sp.tile([NCOL, D], F32)
                    nc.gpsimd.indirect_dma_start(
                        out=rollf[:], out_offset=None, in_=z[:],
                        in_offset=IndirectOffsetOnAxis(ap=bidx_i[:, 0:1], axis=0),
                    )
                    t1b = sp.tile([NCOL, D], F32)
                    nc.scalar.mul(out=t1b[:], in_=upf[:], mul=pb_col[:])
                    ob = sp.tile([NCOL, D], BF16)
                    nc.vector.scalar_tensor_tensor(
                        out=ob[:], in0=rollf[:], scalar=qb_col[:], in1=t1b[:],
                        op0=mybir.AluOpType.mult, op1=mybir.AluOpType.add,
                    )

            # ---- epilogue store: rows t = 128j to their own tensor ---------
            nc.sync.dma_start(out=out2[:, :], in_=ob[:])

    # Run the Bacc lowering passes (register allocation, event-semaphore
    # splitting, ...) — run_bass_via_pjrt serializes nc.m as-is.
    nc.finalize()
    return nc


_NC_CACHE = None


def _get_nc() -> bass.Bass:
    global _NC_CACHE
    if _NC_CACHE is None:
        _NC_CACHE = build_bass()
    return _NC_CACHE


def make_in_maps(z: np.ndarray, p: np.ndarray, b: np.ndarray) -> list[dict]:
    return [
        {
            "z": np.ascontiguousarray(z[i], dtype=np.float32),
            "p": np.ascontiguousarray(p[i], dtype=np.float32),
            "b": np.ascontiguousarray(b[i], dtype=np.int32),
        }
        for i in range(B)
    ]


def kernel(z, p, b, original_len=None, **_unused) -> np.ndarray:
    z = np.asarray(z, dtype=np.float32)
    p = np.asarray(p, dtype=np.float32)
    b = np.asarray(b, dtype=np.int32)
    assert z.shape == (B, NCH, D) and p.shape == (B, T) and b.shape == (B, T)

    nc = _get_nc()
    res = run_bass_kernel_spmd(nc, make_in_maps(z, p, b), list(range(B)))
    outs = []
    for r in res.results:
        full = np.asarray(r["out"]).astype(np.float32)       # [T, D]
        rows0 = np.asarray(r["out2"]).astype(np.float32)     # [NT, D]
        full[0::P, :] = rows0                                # merge t = 128j rows
        outs.append(full)
    return np.stack(outs, axis=0)

